# revision 15
# baseline (speedup 1.0000x reference)
"""Trainium2 Bass kernel for nn_CondenseSFR (BN+ReLU+shuffle+grouped1x1conv+reindex).

Algebra: out = einsum('nchw,cd->ndhw', conv(shuffle(relu(bn(x)))), index).
Everything except the ReLU is linear in the channel dimension, and the BN
scale inv = gamma*rsqrt(var+eps) is strictly positive, so
    relu(inv*x + b) = inv * relu(x + b/inv)
and the shuffle + grouped conv + reindex fold into a single dense 512x512
channel matrix applied after the ReLU:
    out[n,d,s] = sum_c B[d,c] * relu(x[n,c,s] + bprime[c])
with B = (index^T @ A) * inv[None,:],  A the shuffle-permuted block-diagonal
conv weight, bprime = (beta - mean*inv)/inv.

Tolerance is 2e-2 so the whole pipeline runs in bf16 (measured end-to-end
rel err ~3.3e-3): x, weights and the stored output are bf16; matmul
accumulation stays fp32 in PSUM. vs fp32 this halves HBM traffic
(17.8 -> 8.9 MB/core; the fp32 version sat at the ~358 GB/s HBM-per-core
wall) and makes the PE the critical path: 128 N=512 bf16 matmuls/core
stream at 215 ns each (~27.5us).

Schedule notes (from perfetto/ntff traces):
  - the DMA engines come out of reset at ~80 GB/s and only reach ~390
    GB/s after ~5us, so the kernel front-loads the smallest possible
    critical set (bias, ct0 weight block, first half of x image 0) and
    covers the ramp with PE warm-up matmuls (HAM clock gate needs ~3.4us
    of sustained activity before the PE runs at 2.4 GHz)
  - engine queues are in-order: mid-stream the DVE runs relus ONLY (evac
    work would head-of-line-block the next image's relus), ACT runs all
    PSUM evacuation, store triggers live on Sync/SWDGE
  - PSUM is tiled one bank per tile (8x [128,512]): the Tile tracker
    serializes cross-engine access at tile granularity, so per-bank tiles
    let the last image's drain split ACT/DVE without false dependencies
    and decouple evacuation from the next image's accumulation
  - the last image's stores ride the two HWDGE rings (Sync+Scalar) only;
    SWDGE's ~1us first-byte latency and queue drain would stretch the tail
"""

import numpy as np

import concourse.bacc as bacc
import concourse.mybir as mybir
from concourse.tile import TileContext
from concourse.bass_utils import run_bass_kernel_spmd

EPS = 1e-5
GROUPS = 4
N, C, H, W = 32, 512, 32, 32
HW = H * W                 # 1024
NCORES = 8
NPER = N // NCORES         # 4 images per core
CT = C // 128              # 4 channel tiles
F32 = mybir.dt.float32
BF16 = mybir.dt.bfloat16

_NC_CACHE = None


def _build_nc():
    """Build the (SPMD, per-core) Bass program. Same program on all 8 cores."""
    nc = bacc.Bacc(None, enable_partition_id=False)

    x_d = nc.dram_tensor("x", [NPER, CT, 128, HW], BF16, kind="ExternalInput")
    w_d = nc.dram_tensor("w", [CT, 128, CT * 128], BF16, kind="ExternalInput")
    b_d = nc.dram_tensor("b", [128, CT], F32, kind="ExternalInput")
    o_d = nc.dram_tensor("o", [NPER, CT, 128, HW], BF16, kind="ExternalOutput")

    with TileContext(nc) as tc:
        with (
            tc.tile_pool(name="const", bufs=1) as const,
            tc.tile_pool(name="xin", bufs=4) as xin,
            tc.tile_pool(name="act", bufs=3) as actp,
            tc.tile_pool(name="pp", bufs=8, space="PSUM") as pp,
            tc.tile_pool(name="outp", bufs=2) as outp,
        ):
            # Critical-set DMAs first. Scalar ring: bias, then the weight
            # blocks in ct order (the first matmuls need only the ct0
            # block); Sync ring starts on x image 0 (in half-chunks so the
            # first relu fires after 131KB instead of 262KB).
            bt = const.tile([128, CT], F32)
            nc.scalar.dma_start(bt[:], b_d[:])
            wts = []
            for ct in range(CT):
                wt = const.tile([128, CT * 128], BF16, name=f"w{ct}")
                wts.append(wt)
                nc.scalar.dma_start(wt[:], w_d[ct])

            # PE warm-up over zeroed scratch, sized to end right when the
            # first x bytes + relu are ready (~10us); the dummy PSUM tile
            # shares tag ps0 and is released before image 0 needs the bank.
            wu = const.tile([128, 256], BF16)
            nc.vector.memset(wu[:], 0.0)
            wu_ps = pp.tile([128, 512], F32, name="wu_ps", tag="ps0", bufs=1)
            for _ in range(13):
                nc.tensor.matmul(
                    wu_ps[:, :256], wu[:, :128], wu[:, :256],
                    start=True, stop=True,
                )

            # Input chunks alternate the two HWDGE rings, all pre-issued.
            xts = []
            for n in range(NPER):
                xt = xin.tile([128, CT * HW], BF16, name=f"xt{n}", tag="xt")
                xts.append(xt)
                if n == 0:
                    for k in range(2 * CT):
                        ct, h = divmod(k, 2)
                        eng = nc.sync if ct % 2 == 0 else nc.scalar
                        sl = slice(ct * HW + h * 512, ct * HW + (h + 1) * 512)
                        eng.dma_start(xt[:, sl], x_d[n, ct][:, h * 512:(h + 1) * 512])
                else:
                    for ct in range(CT):
                        eng = nc.sync if ct % 2 == 0 else nc.scalar
                        eng.dma_start(xt[:, ct * HW:(ct + 1) * HW], x_d[n, ct])

            for n in range(NPER):
                xt = xts[n]
                ut = actp.tile([128, CT * HW], BF16)
                # one PSUM bank per (d-tile, half): matmuls start as soon as
                # the first channel tile lands (ct-major accumulation), and
                # each bank evacuates the moment its accumulation stops
                pss = [
                    pp.tile([128, 512], F32, name=f"ps_{n}_{j}", tag=f"ps{j}", bufs=1)
                    for j in range(2 * CT)
                ]

                def relu(ct, h2=None):
                    # relu(x + b) on DVE as one fused tensor_scalar(add,max)
                    # (bf16 in/out -> 4x packed mode)
                    sl = (slice(ct * HW, (ct + 1) * HW) if h2 is None
                          else slice(ct * HW + h2 * 512, ct * HW + (h2 + 1) * 512))
                    nc.vector.tensor_scalar(
                        ut[:, sl], xt[:, sl], bt[:, ct:ct + 1], 0.0,
                        mybir.AluOpType.add, mybir.AluOpType.max,
                    )

                for ct in range(CT):
                    if n == 0:
                        relu(ct, 0)
                        relu(ct, 1)
                    else:
                        relu(ct)
                    for dt_ in range(CT):
                        for half in range(2):
                            ucol = ct * HW + half * 512
                            nc.tensor.matmul(
                                pss[2 * dt_ + half][:],
                                wts[ct][:, dt_ * 128:(dt_ + 1) * 128],
                                ut[:, ucol:ucol + 512],
                                start=(ct == 0),
                                stop=(ct == CT - 1),
                            )

                last = n == NPER - 1
                ot = outp.tile([128, CT * HW], BF16)
                for k in range(2 * CT):
                    dt_, h = divmod(k, 2)
                    osl = ot[:, dt_ * HW + h * 512:dt_ * HW + (h + 1) * 512]
                    # evacuation (with the fp32->bf16 cast) on ACT; the last
                    # image splits ACT/DVE - separate per-bank tiles, so no
                    # cross-engine tile serialization
                    if last and h == 1:
                        nc.vector.tensor_copy(osl, pss[k][:])
                    else:
                        nc.scalar.copy(osl, pss[k][:])
                    if last:
                        # half-granular stores on the two HWDGE rings
                        eng = nc.sync if h == 0 else nc.scalar
                        eng.dma_start(o_d[n, dt_][:, h * 512:(h + 1) * 512], osl)
                    elif h == 1:
                        # per-dt stores off the evac engine: Sync ring for
                        # dt odd, SWDGE for dt even
                        eng = nc.sync if dt_ % 2 == 1 else nc.gpsimd
                        eng.dma_start(o_d[n, dt_], ot[:, dt_ * HW:(dt_ + 1) * HW])

    nc.finalize()
    return nc


def _prep_inputs(x, gamma, beta, running_mean, running_var, weight, index):
    """Fold BN/shuffle/conv/index into (per-core x shards, weight matrix)."""
    f64 = np.float64
    x = np.asarray(x)
    gamma = np.asarray(gamma).astype(f64)
    beta = np.asarray(beta).astype(f64)
    mean = np.asarray(running_mean).astype(f64)
    var = np.asarray(running_var).astype(f64)
    weight = np.asarray(weight)
    index = np.asarray(index)
    Wc = weight.reshape(C, C // GROUPS).astype(f64)   # (Cout, Cin_per_group)
    idx = index.astype(f64)

    inv = gamma / np.sqrt(var + EPS)                  # > 0
    beta_term = beta - mean * inv
    inv_safe = np.where(inv != 0.0, inv, 1.0)
    bprime = np.where(inv != 0.0, beta_term / inv_safe, 0.0)

    # A[o, c]: conv-after-shuffle as one 512x512 matrix.
    # shuffled channel g*128 + i comes from original channel i*GROUPS + g.
    A = np.zeros((C, C), dtype=f64)
    o = np.arange(C)
    i = np.arange(C // GROUPS)
    src = i[None, :] * GROUPS + (o[:, None] // (C // GROUPS))  # (512, 128)
    A[o[:, None], src] = Wc

    # out[d] = sum_c B[d,c] relu(x_c + bprime_c);  B = (idx^T @ A) * inv
    # Stationary operand is B^T[c, d] = (A^T @ idx) * inv[:, None]
    BT = (A.T @ idx) * inv[:, None]                   # (c, d)

    bf16 = np.dtype(mybir.dt.np(BF16))

    # w[ct, p, dt*128+j] = BT[ct*128+p, dt*128+j]
    w_host = np.ascontiguousarray(
        BT.reshape(CT, 128, CT * 128)
    ).astype(np.float32).astype(bf16)

    xr = np.ascontiguousarray(
        x.reshape(NCORES, NPER, CT, 128, HW)
    ).astype(bf16)
    b_host = np.ascontiguousarray(
        bprime.astype(np.float32).reshape(CT, 128).T
    )                                                  # (128, CT)
    return [{"x": xr[k], "w": w_host, "b": b_host} for k in range(NCORES)]


def _unpack_output(res):
    out = np.concatenate(
        [np.asarray(res.results[k]["o"]) for k in range(NCORES)], axis=0
    )
    return out.astype(np.float32).reshape(N, C, H, W)


def _run(inputs, trace=False):
    global _NC_CACHE
    if _NC_CACHE is None:
        _NC_CACHE = _build_nc()
    in_maps = _prep_inputs(**inputs)
    res = run_bass_kernel_spmd(_NC_CACHE, in_maps, list(range(NCORES)), trace=trace)
    return _unpack_output(res), res


def kernel(**inputs):
    out, _ = _run(inputs, trace=False)
    return out


# revision 17
# speedup vs baseline: 1.1128x; 1.1128x over previous
"""Trainium2 Bass kernel for nn_CondenseSFR (BN+ReLU+shuffle+grouped1x1conv+reindex).

Algebra: out = einsum('nchw,cd->ndhw', conv(shuffle(relu(bn(x)))), index).
Everything except the ReLU is linear in the channel dimension, and the BN
scale inv = gamma*rsqrt(var+eps) is strictly positive, so
    relu(inv*x + b) = inv * relu(x + b/inv)
and the shuffle + grouped conv + reindex fold into a single dense 512x512
channel matrix applied after the ReLU:
    out[n,d,s] = sum_c B[d,c] * relu(x[n,c,s] + bprime[c])
with B = (index^T @ A) * inv[None,:],  A the shuffle-permuted block-diagonal
conv weight, bprime = (beta - mean*inv)/inv.

Tolerance is 2e-2 so the whole pipeline runs in bf16 (measured end-to-end
rel err ~3.3e-3): x, weights and the stored output are bf16; matmul
accumulation stays fp32 in PSUM. vs fp32 this halves HBM traffic
(17.8 -> 8.9 MB/core; the fp32 version sat at the ~358 GB/s HBM-per-core
wall) and makes the PE the critical path: 128 N=512 bf16 matmuls/core
stream at 215 ns each (~27.5us).

Schedule notes (from perfetto/ntff traces):
  - the DMA engines come out of reset at ~80 GB/s and only reach ~390
    GB/s after ~5us, so the kernel front-loads the smallest possible
    critical set (bias, ct0 weight block, first half of x image 0) and
    covers the ramp with PE warm-up matmuls (HAM clock gate needs ~3.4us
    of sustained activity before the PE runs at 2.4 GHz)
  - engine queues are in-order: mid-stream the DVE runs relus ONLY (evac
    work would head-of-line-block the next image's relus), ACT runs all
    PSUM evacuation, store triggers live on Sync/SWDGE
  - PSUM is tiled one bank per tile (8x [128,512]): the Tile tracker
    serializes cross-engine access at tile granularity, so per-bank tiles
    let the last image's drain split ACT/DVE without false dependencies
    and decouple evacuation from the next image's accumulation
  - the last image's stores ride the two HWDGE rings (Sync+Scalar) only;
    SWDGE's ~1us first-byte latency and queue drain would stretch the tail
"""

import numpy as np

import concourse.bacc as bacc
import concourse.mybir as mybir
from concourse.tile import TileContext
from concourse.bass_utils import run_bass_kernel_spmd

EPS = 1e-5
GROUPS = 4
N, C, H, W = 32, 512, 32, 32
HW = H * W                 # 1024
NCORES = 8
NPER = N // NCORES         # 4 images per core
CT = C // 128              # 4 channel tiles
F32 = mybir.dt.float32
BF16 = mybir.dt.bfloat16

_NC_CACHE = None


def _build_nc():
    """Build the (SPMD, per-core) Bass program. Same program on all 8 cores."""
    nc = bacc.Bacc(None, enable_partition_id=False)

    x_d = nc.dram_tensor("x", [NPER, CT, 128, HW], BF16, kind="ExternalInput")
    w_d = nc.dram_tensor("w", [CT, 128, CT * 128], BF16, kind="ExternalInput")
    b_d = nc.dram_tensor("b", [128, CT], F32, kind="ExternalInput")
    o_d = nc.dram_tensor("o", [NPER, CT, 128, HW], BF16, kind="ExternalOutput")

    with TileContext(nc) as tc:
        with (
            tc.tile_pool(name="const", bufs=1) as const,
            tc.tile_pool(name="xin", bufs=4) as xin,
            tc.tile_pool(name="act", bufs=3) as actp,
            tc.tile_pool(name="pp", bufs=8, space="PSUM") as pp,
            tc.tile_pool(name="outp", bufs=2) as outp,
        ):
            # Critical-set DMAs first, split across the two HWDGE rings so
            # the cold-DMA window (~80 GB/s aggregate for the first ~5us)
            # works on both halves of the first matmul's needs in parallel:
            # Scalar ring: bias then the weight blocks in ct order (the
            # first matmuls need only the ct0 block = 130KB); Sync ring:
            # all of x image 0 (ct0 in half-chunks so the first relu fires
            # after 131KB instead of 262KB).
            bt = const.tile([128, CT], F32)
            nc.scalar.dma_start(bt[:], b_d[:])
            wts = []
            for ct in range(CT):
                wt = const.tile([128, CT * 128], BF16, name=f"w{ct}")
                wts.append(wt)
                nc.scalar.dma_start(wt[:], w_d[ct])

            # PE warm-up over zeroed scratch, sized to end right when the
            # first x bytes + relu are ready (~10.7us) with the HAM clock
            # fully ramped; the dummy PSUM tile shares tag ps0 and is
            # released before image 0 needs the bank.
            wu = const.tile([128, 256], BF16)
            nc.vector.memset(wu[:], 0.0)
            wu_ps = pp.tile([128, 1024], F32, name="wu_ps", tag="ps0", bufs=1)
            for _ in range(16):
                nc.tensor.matmul(
                    wu_ps[:, :256], wu[:, :128], wu[:, :256],
                    start=True, stop=True,
                )

            # Input chunks, all pre-issued. Image 0 entirely on Sync
            # (weights own Scalar early); later images alternate rings.
            xts = []
            for n in range(NPER):
                xt = xin.tile([128, CT * HW], BF16, name=f"xt{n}", tag="xt")
                xts.append(xt)
                if n == 0:
                    for h in range(2):
                        nc.sync.dma_start(
                            xt[:, h * 512:(h + 1) * 512],
                            x_d[n, 0][:, h * 512:(h + 1) * 512],
                        )
                    for ct in range(1, CT):
                        nc.sync.dma_start(xt[:, ct * HW:(ct + 1) * HW], x_d[n, ct])
                else:
                    for ct in range(CT):
                        eng = nc.sync if ct % 2 == 0 else nc.scalar
                        eng.dma_start(xt[:, ct * HW:(ct + 1) * HW], x_d[n, ct])

            for n in range(NPER):
                xt = xts[n]
                ut = actp.tile([128, CT * HW], BF16)
                # 8 PSUM banks accumulate ct-major, so matmuls start as soon
                # as the first channel tile lands instead of after the last.
                # One [128,1024] (2-bank) tile per d-tile: each matmul writes
                # a single bank, but evacuation runs as one FD=1024 copy.
                pss = [
                    pp.tile([128, 1024], F32, name=f"ps_{n}_{j}", tag=f"ps{j}", bufs=1)
                    for j in range(CT)
                ]

                def relu(ct, h2=None):
                    # relu(x + b) on DVE as one fused tensor_scalar(add,max)
                    # (bf16 in/out -> 4x packed mode)
                    sl = (slice(ct * HW, (ct + 1) * HW) if h2 is None
                          else slice(ct * HW + h2 * 512, ct * HW + (h2 + 1) * 512))
                    nc.vector.tensor_scalar(
                        ut[:, sl], xt[:, sl], bt[:, ct:ct + 1], 0.0,
                        mybir.AluOpType.add, mybir.AluOpType.max,
                    )

                for ct in range(CT):
                    if n == 0 and ct == 0:
                        relu(ct, 0)
                        relu(ct, 1)
                    else:
                        relu(ct)
                    for dt_ in range(CT):
                        for half in range(2):
                            ucol = ct * HW + half * 512
                            nc.tensor.matmul(
                                pss[dt_][:, half * 512:(half + 1) * 512],
                                wts[ct][:, dt_ * 128:(dt_ + 1) * 128],
                                ut[:, ucol:ucol + 512],
                                start=(ct == 0),
                                stop=(ct == CT - 1),
                            )

                last = n == NPER - 1
                ot = outp.tile([128, CT * HW], BF16)
                for dt_ in range(CT):
                    osl = ot[:, dt_ * HW:(dt_ + 1) * HW]
                    # PSUM evacuation (with the fp32->bf16 cast) on ACT; the
                    # last image's drain splits across ACT and DVE (whole
                    # tiles - the Tile tracker serializes cross-engine
                    # access at tile granularity)
                    if last and dt_ % 2 == 1:
                        nc.vector.tensor_copy(osl, pss[dt_][:])
                    else:
                        nc.scalar.copy(osl, pss[dt_][:])
                    # stores: dt1/dt3 on the Scalar HWDGE ring (its weight
                    # bytes drain early), dt0/dt2 on SWDGE; the last image
                    # rides both HWDGE rings (SWDGE first-byte latency and
                    # queue drain would stretch the tail)
                    if dt_ % 2 == 1:
                        nc.scalar.dma_start(o_d[n, dt_], osl)
                    elif last:
                        nc.sync.dma_start(o_d[n, dt_], osl)
                    else:
                        nc.gpsimd.dma_start(o_d[n, dt_], osl)

    nc.finalize()
    return nc


def _prep_inputs(x, gamma, beta, running_mean, running_var, weight, index):
    """Fold BN/shuffle/conv/index into (per-core x shards, weight matrix)."""
    f64 = np.float64
    x = np.asarray(x)
    gamma = np.asarray(gamma).astype(f64)
    beta = np.asarray(beta).astype(f64)
    mean = np.asarray(running_mean).astype(f64)
    var = np.asarray(running_var).astype(f64)
    weight = np.asarray(weight)
    index = np.asarray(index)
    Wc = weight.reshape(C, C // GROUPS).astype(f64)   # (Cout, Cin_per_group)
    idx = index.astype(f64)

    inv = gamma / np.sqrt(var + EPS)                  # > 0
    beta_term = beta - mean * inv
    inv_safe = np.where(inv != 0.0, inv, 1.0)
    bprime = np.where(inv != 0.0, beta_term / inv_safe, 0.0)

    # A[o, c]: conv-after-shuffle as one 512x512 matrix.
    # shuffled channel g*128 + i comes from original channel i*GROUPS + g.
    A = np.zeros((C, C), dtype=f64)
    o = np.arange(C)
    i = np.arange(C // GROUPS)
    src = i[None, :] * GROUPS + (o[:, None] // (C // GROUPS))  # (512, 128)
    A[o[:, None], src] = Wc

    # out[d] = sum_c B[d,c] relu(x_c + bprime_c);  B = (idx^T @ A) * inv
    # Stationary operand is B^T[c, d] = (A^T @ idx) * inv[:, None]
    BT = (A.T @ idx) * inv[:, None]                   # (c, d)

    bf16 = np.dtype(mybir.dt.np(BF16))

    # w[ct, p, dt*128+j] = BT[ct*128+p, dt*128+j]
    w_host = np.ascontiguousarray(
        BT.reshape(CT, 128, CT * 128)
    ).astype(np.float32).astype(bf16)

    xr = np.ascontiguousarray(
        x.reshape(NCORES, NPER, CT, 128, HW)
    ).astype(bf16)
    b_host = np.ascontiguousarray(
        bprime.astype(np.float32).reshape(CT, 128).T
    )                                                  # (128, CT)
    return [{"x": xr[k], "w": w_host, "b": b_host} for k in range(NCORES)]


def _unpack_output(res):
    out = np.concatenate(
        [np.asarray(res.results[k]["o"]) for k in range(NCORES)], axis=0
    )
    return out.astype(np.float32).reshape(N, C, H, W)


def _run(inputs, trace=False):
    global _NC_CACHE
    if _NC_CACHE is None:
        _NC_CACHE = _build_nc()
    in_maps = _prep_inputs(**inputs)
    res = run_bass_kernel_spmd(_NC_CACHE, in_maps, list(range(NCORES)), trace=trace)
    return _unpack_output(res), res


def kernel(**inputs):
    out, _ = _run(inputs, trace=False)
    return out
